# revision 10
# baseline (speedup 1.0000x reference)
"""Trainium2 Bass kernel for nn_Attention_21835613733572.

reference:
    score = einsum('bci,bcj->bij', k, q) / sqrt(L)       # (B, L, L)
    score = softmax(score, axis=0)                       # over the BATCH axis
    out   = einsum('bci,bij->bcj', v, score)             # (B, C, L)
with B, C, L = 16, 512, 1024 (f32 inputs/outputs).

Distribution: j-shard over 8 NeuronCores. Each core owns a 128-wide slice of
the j axis, holds the FULL k and the FULL (host-pre-transposed) v in bf16 plus
its q j-slice. Because every core sees all 16 batches for its j-slice, the
batch-axis softmax is core-local - no collectives at all (measured collective
stack latency on this fleet is ~60us before the first AllReduce completes,
which would dominate the kernel).

Per-core pipeline (all matmuls bf16 at N=512):
  MM1:  scoreT[j,i] = sum_c q[c,j] k[c,i]    (lhsT=q stationary, rhs=k)
  exp:  expT = exp(scoreT/sqrt(L))           (ScalarE, fused 1/sqrt(L) scale;
                                              scores are bounded ~+-5 so no
                                              max-subtraction is needed)
  den:  dsumT = sum_b expT[b]                (VectorE pairwise adds, f32)
  rec:  recipT = approx 1/dsumT; transposed to (i,j) via identity matmuls
  prob: probI[b] = transpose(expT[b]) * recip  (PE transpose-matmuls + VectorE)
  MM2:  outT[j,c] = sum_i probI[i,j] vT[i,c] (lhsT=probI stationary, rhs=vT)
Output is outT (j, b, c) bf16 per core; the host re-assembles/un-transposes
and casts back to f32. bf16 end-to-end keeps rel err ~4e-3.

DMA layouts are host-prearranged so every transfer is contiguous per
partition; the kernel is DMA-bound (~36 MB/core at ~320 GB/s).
"""
import sys

sys.path.insert(0, "/opt/trn_rl_repo")

import numpy as np
import ml_dtypes

# Defensive: if BASS_TRACE is set in the environment, run_bass_kernel_spmd's
# axon trace path imports antenv.axon_hooks, which is missing on this image.
# Provide the shim (and a no-op artifact upload) so tracing degrades cleanly.
try:
    import types
    import antenv
    import trn_agent_boot.trn_boot as _tb
    if "antenv.axon_hooks" not in sys.modules:
        _hook = _tb._ntff_profile_via_ctypes("/opt/axon/libaxon_pjrt.so")
        _mod = types.ModuleType("antenv.axon_hooks")
        _mod.get_axon_ntff_profile_hook = lambda: _hook
        _mod.set_axon_ntff_profile_hook = lambda h: None
        sys.modules["antenv.axon_hooks"] = _mod
        antenv.axon_hooks = _mod
except Exception:
    pass

import concourse.bass as bass
import concourse.mybir as mybir
import concourse.tile as tile
from concourse import bacc
from concourse.masks import make_identity
from concourse.bass_utils import run_bass_kernel_spmd

P = 128
B, C, L = 16, 512, 1024
NCORES = 8
J = L // NCORES  # 128
BF16 = mybir.dt.bfloat16
F32 = mybir.dt.float32
CC_N = C // P   # 4
IC_N = L // P   # 8
IH_N = 2        # i-halves of 512 for MM1 psum

_cached_nc = None


def _build():
    nc = bacc.Bacc("TRN2", target_bir_lowering=False, debug=False,
                   num_devices=NCORES)
    q_ext = nc.dram_tensor("q", [P, B, CC_N, J], BF16, kind="ExternalInput").ap()
    k_ext = nc.dram_tensor("k", [P, B, CC_N, L], BF16, kind="ExternalInput").ap()
    vt_ext = nc.dram_tensor("v", [P, B, IC_N, C], BF16, kind="ExternalInput").ap()
    out_ext = nc.dram_tensor("out", [P, B, C], BF16, kind="ExternalOutput").ap()

    with tile.TileContext(nc) as tc:
        with (
            tc.tile_pool(name="const", bufs=1) as const,
            tc.tile_pool(name="qpool", bufs=1) as qpool,
            tc.tile_pool(name="kpool", bufs=14) as kpool,
            tc.tile_pool(name="epool", bufs=4) as epool,
            tc.tile_pool(name="ppool", bufs=1) as ppool,
            tc.tile_pool(name="spool", bufs=1) as spool,
            tc.tile_pool(name="tpool", bufs=6) as tpool,
            tc.tile_pool(name="opool", bufs=7) as opool,
            tc.tile_pool(name="ps1", bufs=3, space="PSUM") as ps1,
            tc.tile_pool(name="pst", bufs=2, space="PSUM") as pst,
            tc.tile_pool(name="ps2", bufs=3, space="PSUM") as ps2,
        ):
            ident = const.tile([P, P], BF16, name="ident")
            make_identity(nc, ident[:])

            # k[0] first so MM1 can start ASAP; q next; k[1:] stream behind.
            # k on the sync HWDGE queue (the binding stream), q and the
            # output DMAs on the ACT HWDGE queue - removes 4 MB from the
            # critical sync stream.
            k_sb = []
            kt0 = kpool.tile([P, CC_N, L], BF16, tag="ktile")
            nc.sync.dma_start(kt0[:], k_ext[:, 0])
            k_sb.append(kt0)
            q_sb = qpool.tile([P, B, CC_N, J], BF16, name="q_all")
            nc.scalar.dma_start(q_sb[:, 0:4], q_ext[:, 0:4])
            nc.scalar.dma_start(q_sb[:, 4:], q_ext[:, 4:])

            # ---- MM1 (scoreT) + exp + transpose-to-(i,j) + denominator ----
            expT, probI = {}, {}
            pend = {}
            dsumT = [spool.tile([P, 512], F32, name=f"dsumT{ih}")
                     for ih in range(IH_N)]
            for b in range(B):
                if b > 0:
                    kt = kpool.tile([P, CC_N, L], BF16, tag="ktile")
                    nc.sync.dma_start(kt[:], k_ext[:, b])
                    k_sb.append(kt)
                kt = k_sb[b]
                et = epool.tile([P, L], BF16, tag="etile")
                for ih in range(IH_N):
                    ps = ps1.tile([P, 512], F32, tag="mm1")
                    for cc in range(CC_N):
                        nc.tensor.matmul(
                            ps[:],
                            q_sb[:, b, cc, :],
                            kt[:, cc, ih * 512:(ih + 1) * 512],
                            start=(cc == 0),
                            stop=(cc == CC_N - 1),
                        )
                    nc.scalar.activation(
                        et[:, ih * 512:(ih + 1) * 512], ps[:],
                        mybir.ActivationFunctionType.Exp,
                        scale=float(1.0 / (L ** 0.5)),
                    )
                expT[b] = et
                # transpose exp blocks to (i, j) with plain matmuls, park in SBUF
                for half in range(2):
                    pt = pst.tile([P, 4, P], F32, tag="ptrans")
                    for s in range(4):
                        ic = half * 4 + s
                        nc.tensor.matmul(
                            pt[:, s], et[:, ic * P:(ic + 1) * P], ident[:],
                            start=True, stop=True,
                        )
                    ei = ppool.tile([P, 4, P], BF16, name=f"probI_{b}_{half}")
                    nc.scalar.copy(ei[:], pt[:])
                    probI[b, half] = ei
                if b % 2 == 1:
                    for ih in range(IH_N):
                        s = tpool.tile([P, 512], F32, tag="lvl0")
                        nc.vector.tensor_add(
                            s[:],
                            expT[b - 1][:, ih * 512:(ih + 1) * 512],
                            et[:, ih * 512:(ih + 1) * 512])
                        if b == 1:
                            pend[ih] = s
                        elif b == 3:
                            nc.vector.tensor_add(dsumT[ih][:], pend[ih][:], s[:])
                        else:
                            nc.vector.tensor_add(dsumT[ih][:], dsumT[ih][:], s[:])

            # ---- reciprocal (approx) + transpose to (i,j) ----
            recipT_bf = spool.tile([P, L], BF16, name="recipT_bf")
            for ih in range(IH_N):
                r = tpool.tile([P, 512], F32, tag="recipT")
                nc.vector.reciprocal_approx_fast(r[:], dsumT[ih][:])
                nc.vector.tensor_copy(recipT_bf[:, ih * 512:(ih + 1) * 512], r[:])
            recip_sb = spool.tile([P, IC_N, P], BF16, name="recip_sb")
            for half in range(2):
                pt = pst.tile([P, 4, P], F32, tag="ptrans")
                for s in range(4):
                    ic = half * 4 + s
                    nc.tensor.matmul(
                        pt[:, s], recipT_bf[:, ic * P:(ic + 1) * P], ident[:],
                        start=True, stop=True,
                    )
                nc.scalar.copy(recip_sb[:, half * 4:(half + 1) * 4, :], pt[:])

            # ---- prob (in place) + MM2 ----
            for b in range(B):
                vt = kpool.tile([P, IC_N, C], BF16, tag="ktile")
                eng = nc.sync if (b % 2 == 0) else nc.scalar
                eng.dma_start(vt[:], vt_ext[:, b])
                for half in range(2):
                    nc.vector.tensor_mul(
                        probI[b, half][:], probI[b, half][:],
                        recip_sb[:, half * 4:(half + 1) * 4, :])
                po = ps2.tile([P, C], F32, tag="mm2")
                for ic in range(IC_N):
                    nc.tensor.matmul(
                        po[:],
                        probI[b, ic // 4][:, ic % 4, :],
                        vt[:, ic, :],
                        start=(ic == 0),
                        stop=(ic == IC_N - 1),
                    )
                ot = opool.tile([P, C], BF16, tag="otile")
                nc.scalar.copy(ot[:], po[:])
                oeng = nc.gpsimd if (b % 2 == 0) else nc.sync
                oeng.dma_start(out_ext[:, b], ot[:])

    nc.compile()
    return nc


def kernel(q: np.ndarray, k: np.ndarray, v: np.ndarray) -> np.ndarray:
    """Full inputs (B, C, L) f32 -> full output (B, C, L) f32."""
    global _cached_nc
    assert q.shape == (B, C, L) and k.shape == (B, C, L) and v.shape == (B, C, L)

    # ---- host prep: bf16 cast, v transpose, contiguous DMA layouts ----
    k_p = np.ascontiguousarray(
        np.asarray(k).astype(ml_dtypes.bfloat16)
        .reshape(B, CC_N, P, L).transpose(2, 0, 1, 3))
    vt = np.ascontiguousarray(np.asarray(v).transpose(0, 2, 1)).astype(
        ml_dtypes.bfloat16)
    v_p = np.ascontiguousarray(vt.reshape(B, IC_N, P, C).transpose(2, 0, 1, 3))
    q_bf = np.asarray(q).astype(ml_dtypes.bfloat16)
    in_maps = []
    for c in range(NCORES):
        qs = q_bf[:, :, c * J:(c + 1) * J]
        q_p = np.ascontiguousarray(
            qs.reshape(B, CC_N, P, J).transpose(2, 0, 1, 3))
        in_maps.append({"q": q_p, "k": k_p, "v": v_p})

    if _cached_nc is None:
        _cached_nc = _build()
    res = run_bass_kernel_spmd(_cached_nc, in_maps, list(range(NCORES)))

    # out param per core: (P, B, C) with out[b, c, j_global] = param[j, b, c]
    out = np.concatenate(
        [np.asarray(res.results[c]["out"]).astype(np.float32).transpose(1, 2, 0)
         for c in range(NCORES)], axis=2)
    return np.ascontiguousarray(out)


if __name__ == "__main__":
    rng = np.random.default_rng(0)
    q = rng.standard_normal((B, C, L)).astype(np.float32)
    k = rng.standard_normal((B, C, L)).astype(np.float32)
    v = rng.standard_normal((B, C, L)).astype(np.float32)
    out = kernel(q=q, k=k, v=v)
    s = np.einsum("bci,bcj->bij", k, q) / np.sqrt(L)
    e = np.exp(s - s.max(axis=0, keepdims=True))
    p = e / e.sum(axis=0, keepdims=True)
    ref = np.einsum("bci,bij->bcj", v, p)
    print("rel fro err:", np.linalg.norm(out - ref) / np.linalg.norm(ref))

